# revision 15
# baseline (speedup 1.0000x reference)
"""Trainium2 Bass kernel for the residual Gaussian filter-bank model.

Model (per sample, 8 sequential banks):
    x_aux_i = x - sum_{k<i} xf_k
    h = relu(maxpool3s2(conv1d(x_aux, w1, s=2)))      # 1->3ch, k=3
    h = relu(maxpool3s2(conv1d(h, w2, s=2)))          # 3->1ch, k=3
    h = mlp(h)  # 36->20->10->1, relu/relu/sigmoid
    f0 = 592*h
    H = exp(-(f - f0)^2 / 50)
    xf = x_aux * H
Outputs: stacked H, xf, x_aux, f0 over banks.

Sharding: pure data parallelism — batch axis split over 8 NeuronCores,
weights replicated. Within a core: tiles of 128 samples on the SBUF
partition axis, frequency axis (592) on the free axis. Tiles are batched
in groups of G=4 along the free axis so each DVE/ACT instruction covers
G tiles (amortizes the ~100-cycle per-op overhead), with 2 groups
pipelined against each other to hide the per-bank MLP latency chain.
"""

import math

import numpy as np

import concourse.bacc as bacc
import concourse.mybir as mybir
import concourse.tile as tile
from concourse.bass_utils import run_bass_kernel_spmd

F = 8          # filter banks
D = 592        # frequency bins
BATCH = 8192
N_CORES = 8
P = 128                      # SBUF partitions (samples per tile)
FILTER_W = 5.0
CC = 1.0 / (2.0 * FILTER_W * FILTER_W)   # 0.02
SQC = math.sqrt(CC)

DT = mybir.dt.float32
AF = mybir.ActivationFunctionType
OP = mybir.AluOpType

# per-bank scalar column layout inside the broadcast "scal" tensor
SCAL_PER_BANK = 22   # 9 conv1_w + 3 conv1_b + 9 conv2_w + 1 conv2_b
N_SCAL = F * SCAL_PER_BANK + 2   # + [296.0, -296*sqrt(c)] bias columns
COL_F0B = F * SCAL_PER_BANK      # 296.0
COL_BSQ = F * SCAL_PER_BANK + 1  # -296*sqrt(c)


def _build(b_local):
    """Build the Bass program for one core processing b_local samples."""
    assert b_local % P == 0
    n_tiles = b_local // P
    G = math.gcd(4, n_tiles)     # tiles per instruction group
    n_grp = n_tiles // G
    nc = bacc.Bacc("TRN2")

    x_l = nc.dram_tensor("x_l", [b_local, D], DT, kind="ExternalInput")
    scal = nc.dram_tensor("scal", [P, N_SCAL], DT, kind="ExternalInput")
    f2 = nc.dram_tensor("f2", [P, D], DT, kind="ExternalInput")
    ident = nc.dram_tensor("ident", [P, P], DT, kind="ExternalInput")
    w1t = nc.dram_tensor("w1t", [36, F * 20], DT, kind="ExternalInput")
    w2t = nc.dram_tensor("w2t", [20, F * 10], DT, kind="ExternalInput")
    w3t = nc.dram_tensor("w3t", [10, F], DT, kind="ExternalInput")
    b1t = nc.dram_tensor("b1t", [20, F], DT, kind="ExternalInput")
    b2t = nc.dram_tensor("b2t", [10, F], DT, kind="ExternalInput")
    b3t = nc.dram_tensor("b3t", [1, F], DT, kind="ExternalInput")  # 0.5*lin3_b

    H_o = nc.dram_tensor("H_o", [F, b_local, D], DT, kind="ExternalOutput")
    xf_o = nc.dram_tensor("xf_o", [F, b_local, D], DT, kind="ExternalOutput")
    xs_o = nc.dram_tensor("xs_o", [F, b_local, D], DT, kind="ExternalOutput")
    f0_o = nc.dram_tensor("f0_o", [F, b_local, 1], DT, kind="ExternalOutput")

    def grp_rows(dram2d, g):
        # [G*P, D] rows of this group, as a [P, G, D] AP matching the SBUF
        # group layout (partition p holds sample t*P+p of each tile t)
        return dram2d[g * G * P:(g + 1) * G * P, :].rearrange(
            "(t p) d -> p t d", p=P)

    with tile.TileContext(nc) as tc:
        with (
            tc.tile_pool(name="const", bufs=1) as cp,
            tc.tile_pool(name="pers", bufs=1) as pers,
            tc.tile_pool(name="xa", bufs=2) as xap,
            tc.tile_pool(name="work", bufs=2) as wp,
            tc.tile_pool(name="big", bufs=2) as bp,
            tc.tile_pool(name="psum", bufs=2, space="PSUM") as pp,
        ):
            scal_s = cp.tile([P, N_SCAL], DT)
            nc.sync.dma_start(scal_s[:], scal[:])
            f2_s = cp.tile([P, D], DT)
            nc.sync.dma_start(f2_s[:], f2[:])
            ident_s = cp.tile([P, P], DT)
            nc.sync.dma_start(ident_s[:], ident[:])
            w1t_s = cp.tile([36, F * 20], DT)
            nc.sync.dma_start(w1t_s[:], w1t[:])
            w2t_s = cp.tile([20, F * 10], DT)
            nc.sync.dma_start(w2t_s[:], w2t[:])
            w3t_s = cp.tile([10, F], DT)
            nc.sync.dma_start(w3t_s[:], w3t[:])
            b1t_s = cp.tile([20, F], DT)
            nc.sync.dma_start(b1t_s[:], b1t[:])
            b2t_s = cp.tile([10, F], DT)
            nc.sync.dma_start(b2t_s[:], b2t[:])
            b3t_s = cp.tile([1, F], DT)
            nc.sync.dma_start(b3t_s[:], b3t[:])

            sc = lambda j: scal_s[:, j:j + 1]  # noqa: E731

            # Per-group persistent state, all groups in flight.
            xgs, f0gs, xauxs = [], [], []
            for g in range(n_grp):
                xg = pers.tile([P, G, D], DT, tag=f"x{g}", name=f"x{g}")
                nc.sync.dma_start(xg[:], grp_rows(x_l, g))
                xgs.append(xg)
                f0gs.append(pers.tile([P, F * G], DT, tag=f"f0g{g}",
                                      name=f"f0g{g}"))
                xauxs.append(xg)

            for i in range(F):
                base = i * SCAL_PER_BANK
                for g in range(n_grp):
                    xaux = xauxs[g]          # [P, G, D]
                    # xs output: the residual fed to this bank
                    nc.sync.dma_start(grp_rows(xs_o[i], g), xaux[:])

                    # conv1: 1->3ch k=3 s=2, 592 -> 295, all G tiles in one
                    # instruction. Tap 0 (w*x+b) on ACT; taps 1,2 on DVE.
                    y1 = wp.tile([P, 3, G, 295], DT, tag="y1")
                    for c in range(3):
                        nc.gpsimd.tensor_scalar(
                            y1[:, c], xaux[:, :, 0:589:2],
                            sc(base + 3 * c), sc(base + 9 + c),
                            OP.mult, OP.add)
                        nc.vector.scalar_tensor_tensor(
                            y1[:, c], xaux[:, :, 1:590:2],
                            sc(base + 3 * c + 1), y1[:, c], OP.mult, OP.add)
                        nc.vector.scalar_tensor_tensor(
                            y1[:, c], xaux[:, :, 2:591:2],
                            sc(base + 3 * c + 2), y1[:, c], OP.mult, OP.add)

                    # maxpool3s2 + relu: 295 -> 147 per channel
                    p1 = wp.tile([P, 3, G, 147], DT, tag="p1")
                    for c in range(3):
                        nc.vector.tensor_tensor(
                            p1[:, c], y1[:, c, :, 1:294:2],
                            y1[:, c, :, 2:295:2], OP.max)
                        nc.vector.scalar_tensor_tensor(
                            p1[:, c], y1[:, c, :, 0:293:2], 0.0, p1[:, c],
                            OP.max, OP.max)

                    # conv2: 3->1ch k=3 s=2, 147 -> 73 (tap (0,0) + bias on
                    # ACT, the other 8 taps accumulate on DVE)
                    y2 = wp.tile([P, G, 73], DT, tag="y2")
                    nc.gpsimd.tensor_scalar(
                        y2[:], p1[:, 0, :, 0:145:2],
                        sc(base + 12), sc(base + 21), OP.mult, OP.add)
                    for c in range(3):
                        for tp in range(3):
                            if c == 0 and tp == 0:
                                continue
                            nc.vector.scalar_tensor_tensor(
                                y2[:], p1[:, c, :, tp:tp + 145:2],
                                sc(base + 12 + 3 * c + tp), y2[:],
                                OP.mult, OP.add)

                    # maxpool3s2 + relu: 73 -> 36
                    h = wp.tile([P, G, 36], DT, tag="h")
                    nc.vector.tensor_tensor(
                        h[:], y2[:, :, 1:72:2], y2[:, :, 2:73:2], OP.max)
                    nc.vector.scalar_tensor_tensor(
                        h[:], y2[:, :, 0:71:2], 0.0, h[:], OP.max, OP.max)

                    # MLP 36->20->10->1 on PE, all G tiles as one N=G*128
                    # moving operand (one PSUM bank at G=4)
                    htg = pp.tile([36, G * P], DT, tag="htg")
                    for t in range(G):
                        nc.tensor.transpose(
                            htg[:, t * P:(t + 1) * P], h[:, t], ident_s[:])
                    hts = wp.tile([36, G * P], DT, tag="hts")
                    nc.scalar.copy(hts[:], htg[:])
                    ps1 = pp.tile([20, G * P], DT, tag="mlp")
                    nc.tensor.matmul(ps1[:], w1t_s[:, i * 20:(i + 1) * 20],
                                     hts[:])
                    s1 = wp.tile([20, G * P], DT, tag="s1")
                    nc.scalar.activation(s1[:], ps1[:], AF.Relu,
                                         bias=b1t_s[:, i:i + 1])
                    ps2 = pp.tile([10, G * P], DT, tag="mlp")
                    nc.tensor.matmul(ps2[:], w2t_s[:, i * 10:(i + 1) * 10],
                                     s1[:])
                    s2 = wp.tile([10, G * P], DT, tag="s2")
                    nc.scalar.activation(s2[:], ps2[:], AF.Relu,
                                         bias=b2t_s[:, i:i + 1])
                    ps3 = pp.tile([1, G * P], DT, tag="mlp")
                    nc.tensor.matmul(ps3[:], w3t_s[:, i:i + 1], s2[:])
                    # sigmoid(z+b) = 0.5*(1 + tanh(0.5*z + 0.5*b)); tanh is in
                    # the same ACT table set as exp/square (no table swap)
                    tz = wp.tile([1, G * P], DT, tag="tz")
                    nc.scalar.activation(tz[:], ps3[:], AF.Tanh,
                                         bias=b3t_s[:1, i:i + 1], scale=0.5)
                    pzt = pp.tile([P, G], DT, tag="pzt")
                    for t in range(G):
                        nc.tensor.transpose(
                            pzt[:, t:t + 1], tz[:, t * P:(t + 1) * P],
                            ident_s[:1, :1])
                    # f0 = 592*sigmoid = 296*(1+tanh); bias for the squared
                    # term: -sqrt(c)*f0. Both read the PSUM column directly.
                    f0g = f0gs[g]
                    bsq = wp.tile([P, G], DT, tag="bsq")
                    for t in range(G):
                        nc.scalar.activation(
                            f0g[:, i * G + t:i * G + t + 1], pzt[:, t:t + 1],
                            AF.Identity, bias=sc(COL_F0B), scale=296.0)
                        nc.scalar.activation(
                            bsq[:, t:t + 1], pzt[:, t:t + 1],
                            AF.Identity, bias=sc(COL_BSQ), scale=-296.0 * SQC)

                    # H = exp(-c*(f-f0)^2): square(scale*f + bias) then
                    # exp(-u); per tile (the bias column differs per tile)
                    ug = bp.tile([P, G, D], DT, tag="u")
                    Hg = bp.tile([P, G, D], DT, tag="H")
                    xfg = bp.tile([P, G, D], DT, tag="xf")
                    nxa = None
                    if i < F - 1:
                        nxa = xap.tile([P, G, D], DT, tag=f"xa{g}",
                                       name=f"xa{g}")
                    # per-tile so xf_t/sub_t (DVE) pipeline against the next
                    # tile's square/exp (ACT)
                    for t in range(G):
                        nc.scalar.activation(ug[:, t], f2_s[:], AF.Square,
                                             bias=bsq[:, t:t + 1], scale=SQC)
                        nc.scalar.activation(Hg[:, t], ug[:, t], AF.Exp,
                                             scale=-1.0)
                        nc.vector.tensor_tensor(xfg[:, t], xaux[:, t],
                                                Hg[:, t], OP.mult)
                        if nxa is not None:
                            nc.vector.tensor_tensor(nxa[:, t], xaux[:, t],
                                                    xfg[:, t], OP.subtract)
                    nc.sync.dma_start(grp_rows(H_o[i], g), Hg[:])
                    nc.sync.dma_start(grp_rows(xf_o[i], g), xfg[:])
                    if nxa is not None:
                        xauxs[g] = nxa

            # f0 outputs: per group transpose [P, F*G] -> [F*G, P]; column
            # order inside f0g is i*G+t, matching the (i, t, p) DMA order.
            for g in range(n_grp):
                pf0 = pp.tile([F * G, P], DT, tag="pf0", bufs=1)
                nc.tensor.transpose(pf0[:], f0gs[g][:], ident_s[:])
                f0r = wp.tile([F * G, P], DT, tag="f0r")
                nc.scalar.copy(f0r[:], pf0[:])
                rs = g * G * P
                for i in range(F):
                    dst = f0_o[i, rs:rs + G * P, 0].rearrange(
                        "(t p) -> t p", p=P)
                    nc.sync.dma_start(dst, f0r[i * G:(i + 1) * G, :])

    nc.compile()
    return nc


def _prep_weights(conv1_w, conv1_b, conv2_w, conv2_b,
                  lin1_w, lin1_b, lin2_w, lin2_b, lin3_w, lin3_b):
    """Host-side packing of the tiny per-bank parameters."""
    f32 = np.float32
    row = np.empty(N_SCAL, f32)
    row[COL_F0B] = 296.0
    row[COL_BSQ] = -296.0 * SQC
    for i in range(F):
        b = i * SCAL_PER_BANK
        row[b:b + 9] = conv1_w[i, :, 0, :].reshape(9)
        row[b + 9:b + 12] = conv1_b[i]
        row[b + 12:b + 21] = conv2_w[i, 0, :, :].reshape(9)
        row[b + 21] = conv2_b[i, 0]
    ins = {
        "scal": np.ascontiguousarray(np.tile(row, (P, 1))),
        "f2": np.ascontiguousarray(
            np.tile(np.arange(D, dtype=f32), (P, 1))),
        "ident": np.eye(P, dtype=f32),
        "w1t": np.ascontiguousarray(
            np.concatenate([lin1_w[i].T for i in range(F)], axis=1)),
        "w2t": np.ascontiguousarray(
            np.concatenate([lin2_w[i].T for i in range(F)], axis=1)),
        "w3t": np.ascontiguousarray(
            np.concatenate([lin3_w[i].T for i in range(F)], axis=1)),
        "b1t": np.ascontiguousarray(lin1_b.T),
        "b2t": np.ascontiguousarray(lin2_b.T),
        "b3t": np.ascontiguousarray(0.5 * lin3_b.T),
    }
    return {k: v.astype(f32, copy=False) for k, v in ins.items()}


_NC_CACHE = {}


def _get_nc(b_local):
    if b_local not in _NC_CACHE:
        _NC_CACHE[b_local] = _build(b_local)
    return _NC_CACHE[b_local]


def kernel(x, conv1_w, conv1_b, conv2_w, conv2_b,
           lin1_w, lin1_b, lin2_w, lin2_b, lin3_w, lin3_b,
           _trace=False, _tmpdir=None):
    x = np.asarray(x, np.float32)
    args = [np.asarray(a, np.float32) for a in
            (conv1_w, conv1_b, conv2_w, conv2_b,
             lin1_w, lin1_b, lin2_w, lin2_b, lin3_w, lin3_b)]
    B = x.shape[0]
    assert B % N_CORES == 0
    b_local = B // N_CORES

    nc = _get_nc(b_local)
    shared = _prep_weights(*args)
    in_maps = [
        dict(shared, x_l=np.ascontiguousarray(x[c * b_local:(c + 1) * b_local]))
        for c in range(N_CORES)
    ]
    kw = {}
    if _trace:
        kw = dict(trace=True, tmpdir=_tmpdir)
    res = run_bass_kernel_spmd(nc, in_maps, core_ids=list(range(N_CORES)), **kw)
    outs = res.results
    H = np.concatenate([r["H_o"] for r in outs], axis=1)
    xf = np.concatenate([r["xf_o"] for r in outs], axis=1)
    xs = np.concatenate([r["xs_o"] for r in outs], axis=1)
    f0 = np.concatenate([r["f0_o"] for r in outs], axis=1)
    kernel.last_exec_time_ns = res.exec_time_ns
    return (H, xf, xs, f0)


# revision 17
# speedup vs baseline: 1.0922x; 1.0922x over previous
"""Trainium2 Bass kernel for the residual Gaussian filter-bank model.

Model (per sample, 8 sequential banks):
    x_aux_i = x - sum_{k<i} xf_k
    h = relu(maxpool3s2(conv1d(x_aux, w1, s=2)))      # 1->3ch, k=3
    h = relu(maxpool3s2(conv1d(h, w2, s=2)))          # 3->1ch, k=3
    h = mlp(h)  # 36->20->10->1, relu/relu/sigmoid
    f0 = 592*h
    H = exp(-(f - f0)^2 / 50)
    xf = x_aux * H
Outputs: stacked H, xf, x_aux, f0 over banks.

Sharding: pure data parallelism — batch axis split over 8 NeuronCores,
weights replicated. Within a core: tiles of 128 samples on the SBUF
partition axis, frequency axis (592) on the free axis. Tiles are batched
in groups of G=4 along the free axis so each DVE/ACT instruction covers
G tiles (amortizes the ~100-cycle per-op overhead), with 2 groups
pipelined against each other to hide the per-bank MLP latency chain.
"""

import math

import numpy as np

import concourse.bacc as bacc
import concourse.mybir as mybir
import concourse.tile as tile
from concourse.bass_utils import run_bass_kernel_spmd

F = 8          # filter banks
D = 592        # frequency bins
BATCH = 8192
N_CORES = 8
P = 128                      # SBUF partitions (samples per tile)
FILTER_W = 5.0
CC = 1.0 / (2.0 * FILTER_W * FILTER_W)   # 0.02
SQC = math.sqrt(CC)

DT = mybir.dt.float32
AF = mybir.ActivationFunctionType
OP = mybir.AluOpType

# per-bank scalar column layout inside the broadcast "scal" tensor
SCAL_PER_BANK = 22   # 9 conv1_w + 3 conv1_b + 9 conv2_w + 1 conv2_b
N_SCAL = F * SCAL_PER_BANK + 2   # + [296.0, -296*sqrt(c)] bias columns
COL_F0B = F * SCAL_PER_BANK      # 296.0
COL_BSQ = F * SCAL_PER_BANK + 1  # -296*sqrt(c)


def _build(b_local):
    """Build the Bass program for one core processing b_local samples."""
    assert b_local % P == 0
    n_tiles = b_local // P
    G = math.gcd(4, n_tiles)     # tiles per instruction group
    n_grp = n_tiles // G
    nc = bacc.Bacc("TRN2")

    x_l = nc.dram_tensor("x_l", [b_local, D], DT, kind="ExternalInput")
    scal = nc.dram_tensor("scal", [P, N_SCAL], DT, kind="ExternalInput")
    f2 = nc.dram_tensor("f2", [P, D], DT, kind="ExternalInput")
    ident = nc.dram_tensor("ident", [P, P], DT, kind="ExternalInput")
    w1t = nc.dram_tensor("w1t", [36, F * 20], DT, kind="ExternalInput")
    w2t = nc.dram_tensor("w2t", [20, F * 10], DT, kind="ExternalInput")
    w3t = nc.dram_tensor("w3t", [10, F], DT, kind="ExternalInput")
    b1t = nc.dram_tensor("b1t", [20, F], DT, kind="ExternalInput")
    b2t = nc.dram_tensor("b2t", [10, F], DT, kind="ExternalInput")
    b3t = nc.dram_tensor("b3t", [1, F], DT, kind="ExternalInput")  # 0.5*lin3_b

    H_o = nc.dram_tensor("H_o", [F, b_local, D], DT, kind="ExternalOutput")
    xf_o = nc.dram_tensor("xf_o", [F, b_local, D], DT, kind="ExternalOutput")
    xs_o = nc.dram_tensor("xs_o", [F, b_local, D], DT, kind="ExternalOutput")
    f0_o = nc.dram_tensor("f0_o", [F, b_local, 1], DT, kind="ExternalOutput")

    def grp_rows(dram2d, g):
        # [G*P, D] rows of this group, as a [P, G, D] AP matching the SBUF
        # group layout (partition p holds sample t*P+p of each tile t)
        return dram2d[g * G * P:(g + 1) * G * P, :].rearrange(
            "(t p) d -> p t d", p=P)

    with tile.TileContext(nc) as tc:
        with (
            tc.tile_pool(name="const", bufs=1) as cp,
            tc.tile_pool(name="pers", bufs=1) as pers,
            tc.tile_pool(name="xa", bufs=2) as xap,
            tc.tile_pool(name="work", bufs=2) as wp,
            tc.tile_pool(name="big", bufs=2) as bp,
            tc.tile_pool(name="psum", bufs=2, space="PSUM") as pp,
        ):
            scal_s = cp.tile([P, N_SCAL], DT)
            nc.sync.dma_start(scal_s[:], scal[:])
            f2_s = cp.tile([P, D], DT)
            nc.sync.dma_start(f2_s[:], f2[:])
            ident_s = cp.tile([P, P], DT)
            nc.sync.dma_start(ident_s[:], ident[:])
            w1t_s = cp.tile([36, F * 20], DT)
            nc.sync.dma_start(w1t_s[:], w1t[:])
            w2t_s = cp.tile([20, F * 10], DT)
            nc.sync.dma_start(w2t_s[:], w2t[:])
            w3t_s = cp.tile([10, F], DT)
            nc.sync.dma_start(w3t_s[:], w3t[:])
            b1t_s = cp.tile([20, F], DT)
            nc.sync.dma_start(b1t_s[:], b1t[:])
            b2t_s = cp.tile([10, F], DT)
            nc.sync.dma_start(b2t_s[:], b2t[:])
            b3t_s = cp.tile([1, F], DT)
            nc.sync.dma_start(b3t_s[:], b3t[:])

            sc = lambda j: scal_s[:, j:j + 1]  # noqa: E731

            # Per-group persistent state, all groups in flight.
            xgs, f0gs, xauxs = [], [], []
            for g in range(n_grp):
                xg = pers.tile([P, G, D], DT, tag=f"x{g}", name=f"x{g}")
                nc.sync.dma_start(xg[:], grp_rows(x_l, g))
                xgs.append(xg)
                f0gs.append(pers.tile([P, F * G], DT, tag=f"f0g{g}",
                                      name=f"f0g{g}"))
                xauxs.append(xg)

            for i in range(F):
                base = i * SCAL_PER_BANK
                # Emit every group's conv1 tap-0 first: ACT executes in
                # order, so these must not sit behind another group's long
                # square/exp tail (that would stall DVE's conv start).
                y1s = []
                for g in range(n_grp):
                    xaux = xauxs[g]
                    nc.sync.dma_start(grp_rows(xs_o[i], g), xaux[:])
                    y1 = wp.tile([P, 3, G, 295], DT, tag=f"y1g{g}",
                                 name=f"y1g{g}", bufs=1)
                    y1s.append(y1)
                    for c in range(3):
                        nc.scalar.activation(
                            y1[:, c], xaux[:, :, 0:589:2], AF.Identity,
                            bias=sc(base + 9 + c), scale=sc(base + 3 * c))
                for g in range(n_grp):
                    xaux = xauxs[g]          # [P, G, D]
                    y1 = y1s[g]
                    for c in range(3):
                        nc.vector.scalar_tensor_tensor(
                            y1[:, c], xaux[:, :, 1:590:2],
                            sc(base + 3 * c + 1), y1[:, c], OP.mult, OP.add)
                        nc.vector.scalar_tensor_tensor(
                            y1[:, c], xaux[:, :, 2:591:2],
                            sc(base + 3 * c + 2), y1[:, c], OP.mult, OP.add)

                    # maxpool3s2 + relu: 295 -> 147 per channel
                    p1 = wp.tile([P, 3, G, 147], DT, tag="p1")
                    for c in range(3):
                        nc.vector.tensor_tensor(
                            p1[:, c], y1[:, c, :, 1:294:2],
                            y1[:, c, :, 2:295:2], OP.max)
                        nc.vector.scalar_tensor_tensor(
                            p1[:, c], y1[:, c, :, 0:293:2], 0.0, p1[:, c],
                            OP.max, OP.max)

                    # conv2: 3->1ch k=3 s=2, 147 -> 73 (tap (0,0) + bias on
                    # ACT, the other 8 taps accumulate on DVE)
                    y2 = wp.tile([P, G, 73], DT, tag="y2")
                    nc.gpsimd.tensor_scalar(
                        y2[:], p1[:, 0, :, 0:145:2],
                        sc(base + 12), sc(base + 21), OP.mult, OP.add)
                    for c in range(3):
                        for tp in range(3):
                            if c == 0 and tp == 0:
                                continue
                            nc.vector.scalar_tensor_tensor(
                                y2[:], p1[:, c, :, tp:tp + 145:2],
                                sc(base + 12 + 3 * c + tp), y2[:],
                                OP.mult, OP.add)

                    # maxpool3s2 + relu: 73 -> 36
                    h = wp.tile([P, G, 36], DT, tag="h")
                    nc.vector.tensor_tensor(
                        h[:], y2[:, :, 1:72:2], y2[:, :, 2:73:2], OP.max)
                    nc.vector.scalar_tensor_tensor(
                        h[:], y2[:, :, 0:71:2], 0.0, h[:], OP.max, OP.max)

                    # MLP 36->20->10->1 on PE, all G tiles as one N=G*128
                    # moving operand (one PSUM bank at G=4)
                    htg = pp.tile([36, G * P], DT, tag="htg")
                    for t in range(G):
                        nc.tensor.transpose(
                            htg[:, t * P:(t + 1) * P], h[:, t], ident_s[:])
                    hts = wp.tile([36, G * P], DT, tag="hts")
                    nc.scalar.copy(hts[:], htg[:])
                    ps1 = pp.tile([20, G * P], DT, tag="mlp")
                    nc.tensor.matmul(ps1[:], w1t_s[:, i * 20:(i + 1) * 20],
                                     hts[:])
                    s1 = wp.tile([20, G * P], DT, tag="s1")
                    nc.scalar.activation(s1[:], ps1[:], AF.Relu,
                                         bias=b1t_s[:, i:i + 1])
                    ps2 = pp.tile([10, G * P], DT, tag="mlp")
                    nc.tensor.matmul(ps2[:], w2t_s[:, i * 10:(i + 1) * 10],
                                     s1[:])
                    s2 = wp.tile([10, G * P], DT, tag="s2")
                    nc.scalar.activation(s2[:], ps2[:], AF.Relu,
                                         bias=b2t_s[:, i:i + 1])
                    ps3 = pp.tile([1, G * P], DT, tag="mlp")
                    nc.tensor.matmul(ps3[:], w3t_s[:, i:i + 1], s2[:])
                    # sigmoid(z+b) = 0.5*(1 + tanh(0.5*z + 0.5*b)); tanh is in
                    # the same ACT table set as exp/square (no table swap)
                    tz = wp.tile([1, G * P], DT, tag="tz")
                    nc.scalar.activation(tz[:], ps3[:], AF.Tanh,
                                         bias=b3t_s[:1, i:i + 1], scale=0.5)
                    pzt = pp.tile([P, G], DT, tag="pzt")
                    for t in range(G):
                        nc.tensor.transpose(
                            pzt[:, t:t + 1], tz[:, t * P:(t + 1) * P],
                            ident_s[:1, :1])
                    # f0 = 592*sigmoid = 296*(1+tanh); bias for the squared
                    # term: -sqrt(c)*f0. Both read the PSUM column directly.
                    f0g = f0gs[g]
                    bsq = wp.tile([P, G], DT, tag="bsq")
                    # H = exp(-c*(f-f0)^2): square(scale*f + bias) then
                    # exp(-u); per tile (the bias column differs per tile)
                    ug = bp.tile([P, G, D], DT, tag="u")
                    Hg = bp.tile([P, G, D], DT, tag="H")
                    xfg = bp.tile([P, G, D], DT, tag="xf")
                    nxa = None
                    if i < F - 1:
                        nxa = xap.tile([P, G, D], DT, tag=f"xa{g}",
                                       name=f"xa{g}")
                    # per-tile so xf_t/sub_t (DVE) pipeline against the next
                    # tile's square/exp (ACT)
                    for t in range(G):
                        nc.scalar.activation(
                            f0g[:, i * G + t:i * G + t + 1], pzt[:, t:t + 1],
                            AF.Identity, bias=sc(COL_F0B), scale=296.0)
                        nc.scalar.activation(
                            bsq[:, t:t + 1], pzt[:, t:t + 1],
                            AF.Identity, bias=sc(COL_BSQ), scale=-296.0 * SQC)
                        nc.scalar.activation(ug[:, t], f2_s[:], AF.Square,
                                             bias=bsq[:, t:t + 1], scale=SQC)
                        nc.scalar.activation(Hg[:, t], ug[:, t], AF.Exp,
                                             scale=-1.0)
                        nc.vector.tensor_tensor(xfg[:, t], xaux[:, t],
                                                Hg[:, t], OP.mult)
                        if nxa is not None:
                            nc.vector.tensor_tensor(nxa[:, t], xaux[:, t],
                                                    xfg[:, t], OP.subtract)
                    nc.sync.dma_start(grp_rows(H_o[i], g), Hg[:])
                    nc.sync.dma_start(grp_rows(xf_o[i], g), xfg[:])
                    if nxa is not None:
                        xauxs[g] = nxa

            # f0 outputs: per group transpose [P, F*G] -> [F*G, P]; column
            # order inside f0g is i*G+t, matching the (i, t, p) DMA order.
            for g in range(n_grp):
                pf0 = pp.tile([F * G, P], DT, tag="pf0", bufs=1)
                nc.tensor.transpose(pf0[:], f0gs[g][:], ident_s[:])
                f0r = wp.tile([F * G, P], DT, tag="f0r")
                nc.scalar.copy(f0r[:], pf0[:])
                rs = g * G * P
                for i in range(F):
                    dst = f0_o[i, rs:rs + G * P, 0].rearrange(
                        "(t p) -> t p", p=P)
                    nc.sync.dma_start(dst, f0r[i * G:(i + 1) * G, :])

    nc.compile()
    return nc


def _prep_weights(conv1_w, conv1_b, conv2_w, conv2_b,
                  lin1_w, lin1_b, lin2_w, lin2_b, lin3_w, lin3_b):
    """Host-side packing of the tiny per-bank parameters."""
    f32 = np.float32
    row = np.empty(N_SCAL, f32)
    row[COL_F0B] = 296.0
    row[COL_BSQ] = -296.0 * SQC
    for i in range(F):
        b = i * SCAL_PER_BANK
        row[b:b + 9] = conv1_w[i, :, 0, :].reshape(9)
        row[b + 9:b + 12] = conv1_b[i]
        row[b + 12:b + 21] = conv2_w[i, 0, :, :].reshape(9)
        row[b + 21] = conv2_b[i, 0]
    ins = {
        "scal": np.ascontiguousarray(np.tile(row, (P, 1))),
        "f2": np.ascontiguousarray(
            np.tile(np.arange(D, dtype=f32), (P, 1))),
        "ident": np.eye(P, dtype=f32),
        "w1t": np.ascontiguousarray(
            np.concatenate([lin1_w[i].T for i in range(F)], axis=1)),
        "w2t": np.ascontiguousarray(
            np.concatenate([lin2_w[i].T for i in range(F)], axis=1)),
        "w3t": np.ascontiguousarray(
            np.concatenate([lin3_w[i].T for i in range(F)], axis=1)),
        "b1t": np.ascontiguousarray(lin1_b.T),
        "b2t": np.ascontiguousarray(lin2_b.T),
        "b3t": np.ascontiguousarray(0.5 * lin3_b.T),
    }
    return {k: v.astype(f32, copy=False) for k, v in ins.items()}


_NC_CACHE = {}


def _get_nc(b_local):
    if b_local not in _NC_CACHE:
        _NC_CACHE[b_local] = _build(b_local)
    return _NC_CACHE[b_local]


def kernel(x, conv1_w, conv1_b, conv2_w, conv2_b,
           lin1_w, lin1_b, lin2_w, lin2_b, lin3_w, lin3_b,
           _trace=False, _tmpdir=None):
    x = np.asarray(x, np.float32)
    args = [np.asarray(a, np.float32) for a in
            (conv1_w, conv1_b, conv2_w, conv2_b,
             lin1_w, lin1_b, lin2_w, lin2_b, lin3_w, lin3_b)]
    B = x.shape[0]
    assert B % N_CORES == 0
    b_local = B // N_CORES

    nc = _get_nc(b_local)
    shared = _prep_weights(*args)
    in_maps = [
        dict(shared, x_l=np.ascontiguousarray(x[c * b_local:(c + 1) * b_local]))
        for c in range(N_CORES)
    ]
    kw = {}
    if _trace:
        kw = dict(trace=True, tmpdir=_tmpdir)
    res = run_bass_kernel_spmd(nc, in_maps, core_ids=list(range(N_CORES)), **kw)
    outs = res.results
    H = np.concatenate([r["H_o"] for r in outs], axis=1)
    xf = np.concatenate([r["xf_o"] for r in outs], axis=1)
    xs = np.concatenate([r["xs_o"] for r in outs], axis=1)
    f0 = np.concatenate([r["f0_o"] for r in outs], axis=1)
    kernel.last_exec_time_ns = res.exec_time_ns
    return (H, xf, xs, f0)


# revision 19
# speedup vs baseline: 1.1295x; 1.0341x over previous
"""Trainium2 Bass kernel for the residual Gaussian filter-bank model.

Model (per sample, 8 sequential banks):
    x_aux_i = x - sum_{k<i} xf_k
    h = relu(maxpool3s2(conv1d(x_aux, w1, s=2)))      # 1->3ch, k=3
    h = relu(maxpool3s2(conv1d(h, w2, s=2)))          # 3->1ch, k=3
    h = mlp(h)  # 36->20->10->1, relu/relu/sigmoid
    f0 = 592*h
    H = exp(-(f - f0)^2 / 50)
    xf = x_aux * H
Outputs: stacked H, xf, x_aux, f0 over banks.

Sharding: pure data parallelism — batch axis split over 8 NeuronCores,
weights replicated. Within a core: tiles of 128 samples on the SBUF
partition axis, frequency axis (592) on the free axis. Tiles are batched
in groups of G=4 along the free axis so each DVE/ACT instruction covers
G tiles (amortizes the ~100-cycle per-op overhead), with 2 groups
pipelined against each other to hide the per-bank MLP latency chain.
"""

import math

import numpy as np

import concourse.bacc as bacc
import concourse.mybir as mybir
import concourse.tile as tile
from concourse.bass_utils import run_bass_kernel_spmd

F = 8          # filter banks
D = 592        # frequency bins
BATCH = 8192
N_CORES = 8
P = 128                      # SBUF partitions (samples per tile)
FILTER_W = 5.0
CC = 1.0 / (2.0 * FILTER_W * FILTER_W)   # 0.02
SQC = math.sqrt(CC)

DT = mybir.dt.float32
AF = mybir.ActivationFunctionType
OP = mybir.AluOpType

# per-bank scalar column layout inside the broadcast "scal" tensor
SCAL_PER_BANK = 22   # 9 conv1_w + 3 conv1_b + 9 conv2_w + 1 conv2_b
N_SCAL = F * SCAL_PER_BANK + 2   # + [296.0, -296*sqrt(c)] bias columns
COL_F0B = F * SCAL_PER_BANK      # 296.0
COL_BSQ = F * SCAL_PER_BANK + 1  # -296*sqrt(c)


def _build(b_local):
    """Build the Bass program for one core processing b_local samples."""
    assert b_local % P == 0
    n_tiles = b_local // P
    G = math.gcd(2, n_tiles)     # tiles per instruction group
    n_grp = n_tiles // G
    nc = bacc.Bacc("TRN2")

    x_l = nc.dram_tensor("x_l", [b_local, D], DT, kind="ExternalInput")
    scal = nc.dram_tensor("scal", [P, N_SCAL], DT, kind="ExternalInput")
    f2 = nc.dram_tensor("f2", [P, D], DT, kind="ExternalInput")
    ident = nc.dram_tensor("ident", [P, P], DT, kind="ExternalInput")
    w1t = nc.dram_tensor("w1t", [36, F * 20], DT, kind="ExternalInput")
    w2t = nc.dram_tensor("w2t", [20, F * 10], DT, kind="ExternalInput")
    w3t = nc.dram_tensor("w3t", [10, F], DT, kind="ExternalInput")
    b1t = nc.dram_tensor("b1t", [20, F], DT, kind="ExternalInput")
    b2t = nc.dram_tensor("b2t", [10, F], DT, kind="ExternalInput")
    b3t = nc.dram_tensor("b3t", [1, F], DT, kind="ExternalInput")  # 0.5*lin3_b

    H_o = nc.dram_tensor("H_o", [F, b_local, D], DT, kind="ExternalOutput")
    xf_o = nc.dram_tensor("xf_o", [F, b_local, D], DT, kind="ExternalOutput")
    xs_o = nc.dram_tensor("xs_o", [F, b_local, D], DT, kind="ExternalOutput")
    f0_o = nc.dram_tensor("f0_o", [F, b_local, 1], DT, kind="ExternalOutput")

    def grp_rows(dram2d, g):
        # [G*P, D] rows of this group, as a [P, G, D] AP matching the SBUF
        # group layout (partition p holds sample t*P+p of each tile t)
        return dram2d[g * G * P:(g + 1) * G * P, :].rearrange(
            "(t p) d -> p t d", p=P)

    with tile.TileContext(nc) as tc:
        with (
            tc.tile_pool(name="const", bufs=1) as cp,
            tc.tile_pool(name="pers", bufs=1) as pers,
            tc.tile_pool(name="xa", bufs=2) as xap,
            tc.tile_pool(name="work", bufs=2) as wp,
            tc.tile_pool(name="big", bufs=3) as bp,
            tc.tile_pool(name="psum", bufs=2, space="PSUM") as pp,
        ):
            scal_s = cp.tile([P, N_SCAL], DT)
            nc.sync.dma_start(scal_s[:], scal[:])
            f2_s = cp.tile([P, D], DT)
            nc.sync.dma_start(f2_s[:], f2[:])
            ident_s = cp.tile([P, P], DT)
            nc.sync.dma_start(ident_s[:], ident[:])
            w1t_s = cp.tile([36, F * 20], DT)
            nc.sync.dma_start(w1t_s[:], w1t[:])
            w2t_s = cp.tile([20, F * 10], DT)
            nc.sync.dma_start(w2t_s[:], w2t[:])
            w3t_s = cp.tile([10, F], DT)
            nc.sync.dma_start(w3t_s[:], w3t[:])
            b1t_s = cp.tile([20, F], DT)
            nc.sync.dma_start(b1t_s[:], b1t[:])
            b2t_s = cp.tile([10, F], DT)
            nc.sync.dma_start(b2t_s[:], b2t[:])
            b3t_s = cp.tile([1, F], DT)
            nc.sync.dma_start(b3t_s[:], b3t[:])

            sc = lambda j: scal_s[:, j:j + 1]  # noqa: E731

            # Per-group persistent state, all groups in flight.
            xgs, f0gs, xauxs = [], [], []
            for g in range(n_grp):
                xg = pers.tile([P, G, D], DT, tag=f"x{g}", name=f"x{g}")
                nc.sync.dma_start(xg[:], grp_rows(x_l, g))
                xgs.append(xg)
                f0gs.append(pers.tile([P, F * G], DT, tag=f"f0g{g}",
                                      name=f"f0g{g}"))
                xauxs.append(xg)

            for i in range(F):
                base = i * SCAL_PER_BANK
                # Emit every group's conv1 tap-0 first: ACT executes in
                # order, so these must not sit behind another group's long
                # square/exp tail (that would stall DVE's conv start).
                y1s = []
                for g in range(n_grp):
                    xaux = xauxs[g]
                    nc.sync.dma_start(grp_rows(xs_o[i], g), xaux[:])
                    y1 = wp.tile([P, 3, G, 295], DT, tag=f"y1g{g}",
                                 name=f"y1g{g}", bufs=1)
                    y1s.append(y1)
                    for c in range(3):
                        nc.scalar.activation(
                            y1[:, c], xaux[:, :, 0:589:2], AF.Identity,
                            bias=sc(base + 9 + c), scale=sc(base + 3 * c))
                for g in range(n_grp):
                    xaux = xauxs[g]          # [P, G, D]
                    y1 = y1s[g]
                    for c in range(3):
                        nc.vector.scalar_tensor_tensor(
                            y1[:, c], xaux[:, :, 1:590:2],
                            sc(base + 3 * c + 1), y1[:, c], OP.mult, OP.add)
                        nc.vector.scalar_tensor_tensor(
                            y1[:, c], xaux[:, :, 2:591:2],
                            sc(base + 3 * c + 2), y1[:, c], OP.mult, OP.add)

                    # maxpool3s2 + relu: 295 -> 147 per channel
                    p1 = wp.tile([P, 3, G, 147], DT, tag="p1")
                    for c in range(3):
                        nc.vector.tensor_tensor(
                            p1[:, c], y1[:, c, :, 1:294:2],
                            y1[:, c, :, 2:295:2], OP.max)
                        nc.vector.scalar_tensor_tensor(
                            p1[:, c], y1[:, c, :, 0:293:2], 0.0, p1[:, c],
                            OP.max, OP.max)

                    # conv2: 3->1ch k=3 s=2, 147 -> 73 (tap (0,0) + bias on
                    # ACT, the other 8 taps accumulate on DVE)
                    y2 = wp.tile([P, G, 73], DT, tag="y2")
                    nc.gpsimd.tensor_scalar(
                        y2[:], p1[:, 0, :, 0:145:2],
                        sc(base + 12), sc(base + 21), OP.mult, OP.add)
                    for c in range(3):
                        for tp in range(3):
                            if c == 0 and tp == 0:
                                continue
                            nc.vector.scalar_tensor_tensor(
                                y2[:], p1[:, c, :, tp:tp + 145:2],
                                sc(base + 12 + 3 * c + tp), y2[:],
                                OP.mult, OP.add)

                    # maxpool3s2 + relu: 73 -> 36
                    h = wp.tile([P, G, 36], DT, tag="h")
                    nc.vector.tensor_tensor(
                        h[:], y2[:, :, 1:72:2], y2[:, :, 2:73:2], OP.max)
                    nc.vector.scalar_tensor_tensor(
                        h[:], y2[:, :, 0:71:2], 0.0, h[:], OP.max, OP.max)

                    # MLP 36->20->10->1 on PE, all G tiles as one N=G*128
                    # moving operand (one PSUM bank at G=4)
                    htg = pp.tile([36, G * P], DT, tag="htg")
                    for t in range(G):
                        nc.tensor.transpose(
                            htg[:, t * P:(t + 1) * P], h[:, t], ident_s[:])
                    hts = wp.tile([36, G * P], DT, tag="hts")
                    nc.scalar.copy(hts[:], htg[:])
                    ps1 = pp.tile([20, G * P], DT, tag="mlp")
                    nc.tensor.matmul(ps1[:], w1t_s[:, i * 20:(i + 1) * 20],
                                     hts[:])
                    s1 = wp.tile([20, G * P], DT, tag="s1")
                    nc.scalar.activation(s1[:], ps1[:], AF.Relu,
                                         bias=b1t_s[:, i:i + 1])
                    ps2 = pp.tile([10, G * P], DT, tag="mlp")
                    nc.tensor.matmul(ps2[:], w2t_s[:, i * 10:(i + 1) * 10],
                                     s1[:])
                    s2 = wp.tile([10, G * P], DT, tag="s2")
                    nc.scalar.activation(s2[:], ps2[:], AF.Relu,
                                         bias=b2t_s[:, i:i + 1])
                    ps3 = pp.tile([1, G * P], DT, tag="mlp")
                    nc.tensor.matmul(ps3[:], w3t_s[:, i:i + 1], s2[:])
                    # sigmoid(z+b) = 0.5*(1 + tanh(0.5*z + 0.5*b)); tanh is in
                    # the same ACT table set as exp/square (no table swap)
                    tz = wp.tile([1, G * P], DT, tag="tz")
                    nc.scalar.activation(tz[:], ps3[:], AF.Tanh,
                                         bias=b3t_s[:1, i:i + 1], scale=0.5)
                    pzt = pp.tile([P, G], DT, tag="pzt")
                    for t in range(G):
                        nc.tensor.transpose(
                            pzt[:, t:t + 1], tz[:, t * P:(t + 1) * P],
                            ident_s[:1, :1])
                    # f0 = 592*sigmoid = 296*(1+tanh); bias for the squared
                    # term: -sqrt(c)*f0. Both read the PSUM column directly.
                    f0g = f0gs[g]
                    bsq = wp.tile([P, G], DT, tag="bsq")
                    # H = exp(-c*(f-f0)^2): square(scale*f + bias) then
                    # exp(-u); per tile (the bias column differs per tile)
                    ug = bp.tile([P, G, D], DT, tag="u")
                    Hg = bp.tile([P, G, D], DT, tag="H")
                    xfg = bp.tile([P, G, D], DT, tag="xf")
                    nxa = None
                    if i < F - 1:
                        nxa = xap.tile([P, G, D], DT, tag=f"xa{g}",
                                       name=f"xa{g}")
                    # per-tile so xf_t/sub_t (DVE) pipeline against the next
                    # tile's square/exp (ACT)
                    for t in range(G):
                        nc.scalar.activation(
                            f0g[:, i * G + t:i * G + t + 1], pzt[:, t:t + 1],
                            AF.Identity, bias=sc(COL_F0B), scale=296.0)
                        nc.scalar.activation(
                            bsq[:, t:t + 1], pzt[:, t:t + 1],
                            AF.Identity, bias=sc(COL_BSQ), scale=-296.0 * SQC)
                        nc.scalar.activation(ug[:, t], f2_s[:], AF.Square,
                                             bias=bsq[:, t:t + 1], scale=SQC)
                        nc.scalar.activation(Hg[:, t], ug[:, t], AF.Exp,
                                             scale=-1.0)
                        nc.vector.tensor_tensor(xfg[:, t], xaux[:, t],
                                                Hg[:, t], OP.mult)
                        if nxa is not None:
                            nc.vector.tensor_tensor(nxa[:, t], xaux[:, t],
                                                    xfg[:, t], OP.subtract)
                    nc.sync.dma_start(grp_rows(H_o[i], g), Hg[:])
                    nc.sync.dma_start(grp_rows(xf_o[i], g), xfg[:])
                    if nxa is not None:
                        xauxs[g] = nxa

            # f0 outputs: per group transpose [P, F*G] -> [F*G, P]; column
            # order inside f0g is i*G+t, matching the (i, t, p) DMA order.
            for g in range(n_grp):
                pf0 = pp.tile([F * G, P], DT, tag="pf0", bufs=1)
                nc.tensor.transpose(pf0[:], f0gs[g][:], ident_s[:])
                f0r = wp.tile([F * G, P], DT, tag="f0r")
                nc.scalar.copy(f0r[:], pf0[:])
                rs = g * G * P
                for i in range(F):
                    dst = f0_o[i, rs:rs + G * P, 0].rearrange(
                        "(t p) -> t p", p=P)
                    nc.sync.dma_start(dst, f0r[i * G:(i + 1) * G, :])

    nc.compile()
    return nc


def _prep_weights(conv1_w, conv1_b, conv2_w, conv2_b,
                  lin1_w, lin1_b, lin2_w, lin2_b, lin3_w, lin3_b):
    """Host-side packing of the tiny per-bank parameters."""
    f32 = np.float32
    row = np.empty(N_SCAL, f32)
    row[COL_F0B] = 296.0
    row[COL_BSQ] = -296.0 * SQC
    for i in range(F):
        b = i * SCAL_PER_BANK
        row[b:b + 9] = conv1_w[i, :, 0, :].reshape(9)
        row[b + 9:b + 12] = conv1_b[i]
        row[b + 12:b + 21] = conv2_w[i, 0, :, :].reshape(9)
        row[b + 21] = conv2_b[i, 0]
    ins = {
        "scal": np.ascontiguousarray(np.tile(row, (P, 1))),
        "f2": np.ascontiguousarray(
            np.tile(np.arange(D, dtype=f32), (P, 1))),
        "ident": np.eye(P, dtype=f32),
        "w1t": np.ascontiguousarray(
            np.concatenate([lin1_w[i].T for i in range(F)], axis=1)),
        "w2t": np.ascontiguousarray(
            np.concatenate([lin2_w[i].T for i in range(F)], axis=1)),
        "w3t": np.ascontiguousarray(
            np.concatenate([lin3_w[i].T for i in range(F)], axis=1)),
        "b1t": np.ascontiguousarray(lin1_b.T),
        "b2t": np.ascontiguousarray(lin2_b.T),
        "b3t": np.ascontiguousarray(0.5 * lin3_b.T),
    }
    return {k: v.astype(f32, copy=False) for k, v in ins.items()}


_NC_CACHE = {}


def _get_nc(b_local):
    if b_local not in _NC_CACHE:
        _NC_CACHE[b_local] = _build(b_local)
    return _NC_CACHE[b_local]


def kernel(x, conv1_w, conv1_b, conv2_w, conv2_b,
           lin1_w, lin1_b, lin2_w, lin2_b, lin3_w, lin3_b,
           _trace=False, _tmpdir=None):
    x = np.asarray(x, np.float32)
    args = [np.asarray(a, np.float32) for a in
            (conv1_w, conv1_b, conv2_w, conv2_b,
             lin1_w, lin1_b, lin2_w, lin2_b, lin3_w, lin3_b)]
    B = x.shape[0]
    assert B % N_CORES == 0
    b_local = B // N_CORES

    nc = _get_nc(b_local)
    shared = _prep_weights(*args)
    in_maps = [
        dict(shared, x_l=np.ascontiguousarray(x[c * b_local:(c + 1) * b_local]))
        for c in range(N_CORES)
    ]
    kw = {}
    if _trace:
        kw = dict(trace=True, tmpdir=_tmpdir)
    res = run_bass_kernel_spmd(nc, in_maps, core_ids=list(range(N_CORES)), **kw)
    outs = res.results
    H = np.concatenate([r["H_o"] for r in outs], axis=1)
    xf = np.concatenate([r["xf_o"] for r in outs], axis=1)
    xs = np.concatenate([r["xs_o"] for r in outs], axis=1)
    f0 = np.concatenate([r["f0_o"] for r in outs], axis=1)
    kernel.last_exec_time_ns = res.exec_time_ns
    return (H, xf, xs, f0)


# revision 20
# speedup vs baseline: 1.1309x; 1.0013x over previous
"""Trainium2 Bass kernel for the residual Gaussian filter-bank model.

Model (per sample, 8 sequential banks):
    x_aux_i = x - sum_{k<i} xf_k
    h = relu(maxpool3s2(conv1d(x_aux, w1, s=2)))      # 1->3ch, k=3
    h = relu(maxpool3s2(conv1d(h, w2, s=2)))          # 3->1ch, k=3
    h = mlp(h)  # 36->20->10->1, relu/relu/sigmoid
    f0 = 592*h
    H = exp(-(f - f0)^2 / 50)
    xf = x_aux * H
Outputs: stacked H, xf, x_aux, f0 over banks.

Sharding: pure data parallelism — batch axis split over 8 NeuronCores,
weights replicated. Within a core: tiles of 128 samples on the SBUF
partition axis, frequency axis (592) on the free axis. Tiles are batched
in groups of G=4 along the free axis so each DVE/ACT instruction covers
G tiles (amortizes the ~100-cycle per-op overhead), with 2 groups
pipelined against each other to hide the per-bank MLP latency chain.
"""

import math

import numpy as np

import concourse.bacc as bacc
import concourse.mybir as mybir
import concourse.tile as tile
from concourse.bass_utils import run_bass_kernel_spmd

F = 8          # filter banks
D = 592        # frequency bins
BATCH = 8192
N_CORES = 8
P = 128                      # SBUF partitions (samples per tile)
FILTER_W = 5.0
CC = 1.0 / (2.0 * FILTER_W * FILTER_W)   # 0.02
SQC = math.sqrt(CC)

DT = mybir.dt.float32
AF = mybir.ActivationFunctionType
OP = mybir.AluOpType

# per-bank scalar column layout inside the broadcast "scal" tensor
SCAL_PER_BANK = 22   # 9 conv1_w + 3 conv1_b + 9 conv2_w + 1 conv2_b
N_SCAL = F * SCAL_PER_BANK + 2   # + [296.0, -296*sqrt(c)] bias columns
COL_F0B = F * SCAL_PER_BANK      # 296.0
COL_BSQ = F * SCAL_PER_BANK + 1  # -296*sqrt(c)


def _build(b_local):
    """Build the Bass program for one core processing b_local samples."""
    assert b_local % P == 0
    n_tiles = b_local // P
    G = math.gcd(2, n_tiles)     # tiles per instruction group
    n_grp = n_tiles // G
    nc = bacc.Bacc("TRN2")

    x_l = nc.dram_tensor("x_l", [b_local, D], DT, kind="ExternalInput")
    scal = nc.dram_tensor("scal", [P, N_SCAL], DT, kind="ExternalInput")
    f2 = nc.dram_tensor("f2", [P, D], DT, kind="ExternalInput")
    ident = nc.dram_tensor("ident", [P, P], DT, kind="ExternalInput")
    w1t = nc.dram_tensor("w1t", [36, F * 20], DT, kind="ExternalInput")
    w2t = nc.dram_tensor("w2t", [20, F * 10], DT, kind="ExternalInput")
    w3t = nc.dram_tensor("w3t", [10, F], DT, kind="ExternalInput")
    b1t = nc.dram_tensor("b1t", [20, F], DT, kind="ExternalInput")
    b2t = nc.dram_tensor("b2t", [10, F], DT, kind="ExternalInput")
    b3t = nc.dram_tensor("b3t", [1, F], DT, kind="ExternalInput")  # 0.5*lin3_b

    H_o = nc.dram_tensor("H_o", [F, b_local, D], DT, kind="ExternalOutput")
    xf_o = nc.dram_tensor("xf_o", [F, b_local, D], DT, kind="ExternalOutput")
    xs_o = nc.dram_tensor("xs_o", [F, b_local, D], DT, kind="ExternalOutput")
    f0_o = nc.dram_tensor("f0_o", [F, b_local, 1], DT, kind="ExternalOutput")

    def grp_rows(dram2d, g):
        # [G*P, D] rows of this group, as a [P, G, D] AP matching the SBUF
        # group layout (partition p holds sample t*P+p of each tile t)
        return dram2d[g * G * P:(g + 1) * G * P, :].rearrange(
            "(t p) d -> p t d", p=P)

    with tile.TileContext(nc) as tc:
        with (
            tc.tile_pool(name="const", bufs=1) as cp,
            tc.tile_pool(name="pers", bufs=1) as pers,
            tc.tile_pool(name="xa", bufs=2) as xap,
            tc.tile_pool(name="work", bufs=3) as wp,
            tc.tile_pool(name="big", bufs=4) as bp,
            tc.tile_pool(name="psum", bufs=2, space="PSUM") as pp,
        ):
            scal_s = cp.tile([P, N_SCAL], DT)
            nc.sync.dma_start(scal_s[:], scal[:])
            f2_s = cp.tile([P, D], DT)
            nc.sync.dma_start(f2_s[:], f2[:])
            ident_s = cp.tile([P, P], DT)
            nc.sync.dma_start(ident_s[:], ident[:])
            w1t_s = cp.tile([36, F * 20], DT)
            nc.sync.dma_start(w1t_s[:], w1t[:])
            w2t_s = cp.tile([20, F * 10], DT)
            nc.sync.dma_start(w2t_s[:], w2t[:])
            w3t_s = cp.tile([10, F], DT)
            nc.sync.dma_start(w3t_s[:], w3t[:])
            b1t_s = cp.tile([20, F], DT)
            nc.sync.dma_start(b1t_s[:], b1t[:])
            b2t_s = cp.tile([10, F], DT)
            nc.sync.dma_start(b2t_s[:], b2t[:])
            b3t_s = cp.tile([1, F], DT)
            nc.sync.dma_start(b3t_s[:], b3t[:])

            sc = lambda j: scal_s[:, j:j + 1]  # noqa: E731

            # Per-group persistent state, all groups in flight.
            xgs, f0gs, xauxs = [], [], []
            for g in range(n_grp):
                xg = pers.tile([P, G, D], DT, tag=f"x{g}", name=f"x{g}")
                nc.sync.dma_start(xg[:], grp_rows(x_l, g))
                xgs.append(xg)
                f0gs.append(pers.tile([P, F * G], DT, tag=f"f0g{g}",
                                      name=f"f0g{g}"))
                xauxs.append(xg)

            for i in range(F):
                base = i * SCAL_PER_BANK
                # Emit every group's conv1 tap-0 first: ACT executes in
                # order, so these must not sit behind another group's long
                # square/exp tail (that would stall DVE's conv start).
                y1s = []
                for g in range(n_grp):
                    xaux = xauxs[g]
                    nc.sync.dma_start(grp_rows(xs_o[i], g), xaux[:])
                    y1 = wp.tile([P, 3, G, 295], DT, tag=f"y1g{g}",
                                 name=f"y1g{g}", bufs=1)
                    y1s.append(y1)
                    for c in range(3):
                        nc.scalar.activation(
                            y1[:, c], xaux[:, :, 0:589:2], AF.Identity,
                            bias=sc(base + 9 + c), scale=sc(base + 3 * c))
                for g in range(n_grp):
                    xaux = xauxs[g]          # [P, G, D]
                    y1 = y1s[g]
                    for c in range(3):
                        nc.vector.scalar_tensor_tensor(
                            y1[:, c], xaux[:, :, 1:590:2],
                            sc(base + 3 * c + 1), y1[:, c], OP.mult, OP.add)
                        nc.vector.scalar_tensor_tensor(
                            y1[:, c], xaux[:, :, 2:591:2],
                            sc(base + 3 * c + 2), y1[:, c], OP.mult, OP.add)

                    # maxpool3s2 + relu: 295 -> 147 per channel
                    p1 = wp.tile([P, 3, G, 147], DT, tag="p1")
                    for c in range(3):
                        nc.vector.tensor_tensor(
                            p1[:, c], y1[:, c, :, 1:294:2],
                            y1[:, c, :, 2:295:2], OP.max)
                        nc.vector.scalar_tensor_tensor(
                            p1[:, c], y1[:, c, :, 0:293:2], 0.0, p1[:, c],
                            OP.max, OP.max)

                    # conv2: 3->1ch k=3 s=2, 147 -> 73 (tap (0,0) + bias on
                    # ACT, the other 8 taps accumulate on DVE)
                    y2 = wp.tile([P, G, 73], DT, tag="y2")
                    nc.gpsimd.tensor_scalar(
                        y2[:], p1[:, 0, :, 0:145:2],
                        sc(base + 12), sc(base + 21), OP.mult, OP.add)
                    for c in range(3):
                        for tp in range(3):
                            if c == 0 and tp == 0:
                                continue
                            nc.vector.scalar_tensor_tensor(
                                y2[:], p1[:, c, :, tp:tp + 145:2],
                                sc(base + 12 + 3 * c + tp), y2[:],
                                OP.mult, OP.add)

                    # maxpool3s2 + relu: 73 -> 36
                    h = wp.tile([P, G, 36], DT, tag="h")
                    nc.vector.tensor_tensor(
                        h[:], y2[:, :, 1:72:2], y2[:, :, 2:73:2], OP.max)
                    nc.vector.scalar_tensor_tensor(
                        h[:], y2[:, :, 0:71:2], 0.0, h[:], OP.max, OP.max)

                    # MLP 36->20->10->1 on PE, all G tiles as one N=G*128
                    # moving operand (one PSUM bank at G=4)
                    htg = pp.tile([36, G * P], DT, tag="htg")
                    for t in range(G):
                        nc.tensor.transpose(
                            htg[:, t * P:(t + 1) * P], h[:, t], ident_s[:])
                    hts = wp.tile([36, G * P], DT, tag="hts")
                    nc.scalar.copy(hts[:], htg[:])
                    ps1 = pp.tile([20, G * P], DT, tag="mlp")
                    nc.tensor.matmul(ps1[:], w1t_s[:, i * 20:(i + 1) * 20],
                                     hts[:])
                    s1 = wp.tile([20, G * P], DT, tag="s1")
                    nc.scalar.activation(s1[:], ps1[:], AF.Relu,
                                         bias=b1t_s[:, i:i + 1])
                    ps2 = pp.tile([10, G * P], DT, tag="mlp")
                    nc.tensor.matmul(ps2[:], w2t_s[:, i * 10:(i + 1) * 10],
                                     s1[:])
                    s2 = wp.tile([10, G * P], DT, tag="s2")
                    nc.scalar.activation(s2[:], ps2[:], AF.Relu,
                                         bias=b2t_s[:, i:i + 1])
                    ps3 = pp.tile([1, G * P], DT, tag="mlp")
                    nc.tensor.matmul(ps3[:], w3t_s[:, i:i + 1], s2[:])
                    # sigmoid(z+b) = 0.5*(1 + tanh(0.5*z + 0.5*b)); tanh is in
                    # the same ACT table set as exp/square (no table swap)
                    tz = wp.tile([1, G * P], DT, tag="tz")
                    nc.scalar.activation(tz[:], ps3[:], AF.Tanh,
                                         bias=b3t_s[:1, i:i + 1], scale=0.5)
                    pzt = pp.tile([P, G], DT, tag="pzt")
                    for t in range(G):
                        nc.tensor.transpose(
                            pzt[:, t:t + 1], tz[:, t * P:(t + 1) * P],
                            ident_s[:1, :1])
                    # f0 = 592*sigmoid = 296*(1+tanh); bias for the squared
                    # term: -sqrt(c)*f0. Both read the PSUM column directly.
                    f0g = f0gs[g]
                    bsq = wp.tile([P, G], DT, tag="bsq")
                    # H = exp(-c*(f-f0)^2): square(scale*f + bias) then
                    # exp(-u); per tile (the bias column differs per tile)
                    ug = bp.tile([P, G, D], DT, tag="u")
                    Hg = bp.tile([P, G, D], DT, tag="H")
                    xfg = bp.tile([P, G, D], DT, tag="xf")
                    nxa = None
                    if i < F - 1:
                        nxa = xap.tile([P, G, D], DT, tag=f"xa{g}",
                                       name=f"xa{g}")
                    # per-tile so xf_t/sub_t (DVE) pipeline against the next
                    # tile's square/exp (ACT)
                    for t in range(G):
                        nc.scalar.activation(
                            f0g[:, i * G + t:i * G + t + 1], pzt[:, t:t + 1],
                            AF.Identity, bias=sc(COL_F0B), scale=296.0)
                        nc.scalar.activation(
                            bsq[:, t:t + 1], pzt[:, t:t + 1],
                            AF.Identity, bias=sc(COL_BSQ), scale=-296.0 * SQC)
                        nc.scalar.activation(ug[:, t], f2_s[:], AF.Square,
                                             bias=bsq[:, t:t + 1], scale=SQC)
                        nc.scalar.activation(Hg[:, t], ug[:, t], AF.Exp,
                                             scale=-1.0)
                        nc.vector.tensor_tensor(xfg[:, t], xaux[:, t],
                                                Hg[:, t], OP.mult)
                        if nxa is not None:
                            nc.vector.tensor_tensor(nxa[:, t], xaux[:, t],
                                                    xfg[:, t], OP.subtract)
                    nc.sync.dma_start(grp_rows(H_o[i], g), Hg[:])
                    nc.sync.dma_start(grp_rows(xf_o[i], g), xfg[:])
                    if nxa is not None:
                        xauxs[g] = nxa

            # f0 outputs: per group transpose [P, F*G] -> [F*G, P]; column
            # order inside f0g is i*G+t, matching the (i, t, p) DMA order.
            for g in range(n_grp):
                pf0 = pp.tile([F * G, P], DT, tag="pf0", bufs=1)
                nc.tensor.transpose(pf0[:], f0gs[g][:], ident_s[:])
                f0r = wp.tile([F * G, P], DT, tag="f0r")
                nc.scalar.copy(f0r[:], pf0[:])
                rs = g * G * P
                for i in range(F):
                    dst = f0_o[i, rs:rs + G * P, 0].rearrange(
                        "(t p) -> t p", p=P)
                    nc.sync.dma_start(dst, f0r[i * G:(i + 1) * G, :])

    nc.compile()
    return nc


def _prep_weights(conv1_w, conv1_b, conv2_w, conv2_b,
                  lin1_w, lin1_b, lin2_w, lin2_b, lin3_w, lin3_b):
    """Host-side packing of the tiny per-bank parameters."""
    f32 = np.float32
    row = np.empty(N_SCAL, f32)
    row[COL_F0B] = 296.0
    row[COL_BSQ] = -296.0 * SQC
    for i in range(F):
        b = i * SCAL_PER_BANK
        row[b:b + 9] = conv1_w[i, :, 0, :].reshape(9)
        row[b + 9:b + 12] = conv1_b[i]
        row[b + 12:b + 21] = conv2_w[i, 0, :, :].reshape(9)
        row[b + 21] = conv2_b[i, 0]
    ins = {
        "scal": np.ascontiguousarray(np.tile(row, (P, 1))),
        "f2": np.ascontiguousarray(
            np.tile(np.arange(D, dtype=f32), (P, 1))),
        "ident": np.eye(P, dtype=f32),
        "w1t": np.ascontiguousarray(
            np.concatenate([lin1_w[i].T for i in range(F)], axis=1)),
        "w2t": np.ascontiguousarray(
            np.concatenate([lin2_w[i].T for i in range(F)], axis=1)),
        "w3t": np.ascontiguousarray(
            np.concatenate([lin3_w[i].T for i in range(F)], axis=1)),
        "b1t": np.ascontiguousarray(lin1_b.T),
        "b2t": np.ascontiguousarray(lin2_b.T),
        "b3t": np.ascontiguousarray(0.5 * lin3_b.T),
    }
    return {k: v.astype(f32, copy=False) for k, v in ins.items()}


_NC_CACHE = {}


def _get_nc(b_local):
    if b_local not in _NC_CACHE:
        _NC_CACHE[b_local] = _build(b_local)
    return _NC_CACHE[b_local]


def kernel(x, conv1_w, conv1_b, conv2_w, conv2_b,
           lin1_w, lin1_b, lin2_w, lin2_b, lin3_w, lin3_b,
           _trace=False, _tmpdir=None):
    x = np.asarray(x, np.float32)
    args = [np.asarray(a, np.float32) for a in
            (conv1_w, conv1_b, conv2_w, conv2_b,
             lin1_w, lin1_b, lin2_w, lin2_b, lin3_w, lin3_b)]
    B = x.shape[0]
    assert B % N_CORES == 0
    b_local = B // N_CORES

    nc = _get_nc(b_local)
    shared = _prep_weights(*args)
    in_maps = [
        dict(shared, x_l=np.ascontiguousarray(x[c * b_local:(c + 1) * b_local]))
        for c in range(N_CORES)
    ]
    kw = {}
    if _trace:
        kw = dict(trace=True, tmpdir=_tmpdir)
    res = run_bass_kernel_spmd(nc, in_maps, core_ids=list(range(N_CORES)), **kw)
    outs = res.results
    H = np.concatenate([r["H_o"] for r in outs], axis=1)
    xf = np.concatenate([r["xf_o"] for r in outs], axis=1)
    xs = np.concatenate([r["xs_o"] for r in outs], axis=1)
    f0 = np.concatenate([r["f0_o"] for r in outs], axis=1)
    kernel.last_exec_time_ns = res.exec_time_ns
    return (H, xf, xs, f0)
